# revision 36
# baseline (speedup 1.0000x reference)
"""Cross-attention (GQA) Trainium2 Bass kernel, v3.

Problem: B=2, Tq=Tkv=2048, D_MODEL=1024, 16 query heads / 4 kv heads,
head_dim=64.  Sharded over 8 NeuronCores as batch(2) x kv-group(4); each
core computes 4 query heads + its single kv head and a partial output
projection (Wo row-split by head group); partials are summed on host.

Design (v3):
  * single-head (blk x head) iterations with a pipelined B->exp->D
    schedule; exp is true f16 exp on ACT (the DVE bit-trick exp and fp8
    P@V measured 1.8-2.9e-2 max-rel-err -- peaked softmax columns expose
    P's relative quantization directly -- vs the 2e-2 gate, so they're
    disabled).  Steady state is ACT-paced at ~1.0us/tile.
  * scores carry a constant offset row (stationary row 64 = C, moving
    row 64 = 1) kept from the bit-exp experiments; ACT's free bias
    compensates.  Numerically neutral, it keeps exp outputs in a safe
    f16 range for any score outliers.
  * softmax denominators from ones-columns in the P@V stationary;
    reciprocal_approx_fast (only on partitions 0-63: its custom-DVE
    uops are broken at nonzero partition offsets).
  * input DMAs use full-row 2KB descriptors (descriptor count, not
    bytes, bounds DMA: ~57ns/descriptor/queue), split in tq-halves
    across queues; DMA dispatches are spread over the sync, gpsimd and
    vector sequencers (each DIRECT2D dispatch costs ~0.6us serialized
    on its issuing engine's sequencer).
  * E (output projection) for blk0 is interleaved into blk1's
    iterations; output yT is f16 (host accumulates in f32).
  * PSUM: scores pool 2x2 banks, accumulator pool 2x2 banks; PE warmup
    ramps the DVFS p-state (1.2 -> 2.4 GHz) before the first projection.
"""

import os
import sys

import numpy as np

for _p in ("/opt/trn_rl_repo",):
    if _p not in sys.path and os.path.isdir(_p):
        sys.path.insert(0, _p)

import concourse.bass as bass
import concourse.bacc as bacc
import concourse.mybir as mybir
from concourse.tile import TileContext

# ---------------------------------------------------------------- problem dims
B = 2
TQ = 2048
TKV = 2048
D_MODEL = 1024
N_HEADS = 16
N_KV_HEADS = 4
HEAD_DIM = 64
N_CORES = 8
GROUPS = N_KV_HEADS  # kv groups = 4
HEADS_PER_DEV = N_HEADS // GROUPS  # 4
DQ = HEADS_PER_DEV * HEAD_DIM  # 256
DKV = 2 * HEAD_DIM  # 128 (k rows + v rows stacked)
SCALE = 1.0 / float(np.sqrt(HEAD_DIM))

P = 128
FREE = 512  # matmul moving-operand chunk / psum bank width
BLK = 1024  # tq block width

F32 = mybir.dt.float32
F16 = mybir.dt.float16

DT = D_MODEL // P  # 8 d-tiles
ET = DQ // P  # 2 e-tiles (query head pairs)
NCH = TQ // FREE  # 4 chunks of 512
NTK = TKV // P  # 16 tk tiles
NBLK = TQ // BLK  # 2 tq blocks
JPB = BLK // FREE  # 2 free-chunks per block
MT = D_MODEL // P  # 8 output m-tiles

# exp scale handling: scores carry +C_DEV via an offset row; ACT's bias
# restores e^(s/8 - ln2*(15 - AEXP*C/1024)) -- a benign global scale that
# cancels in the softmax normalization.
SHIFT = 3.75
AEXP = 1024.0 * 1.4426950408889634 / 8.0
_LOG2E = 1.4426950408889634
C_OFFSET = (15360.0 - 44.5 - SHIFT * 1024.0 * _LOG2E) / AEXP
C_DEV = float(np.float16(C_OFFSET))
ACT_BIAS = float(np.log(2.0) * (AEXP * C_DEV / 1024.0 - 15.0) - C_DEV / 8.0)


def build_bass():
    nc = bacc.Bacc()

    xq = nc.declare_dram_parameter("xqT", [D_MODEL, TQ], F16, isOutput=False)
    xc = nc.declare_dram_parameter("xcT", [D_MODEL, TKV], F16, isOutput=False)
    wq = nc.declare_dram_parameter("wqT", [D_MODEL, DQ], F16, isOutput=False)
    wkv = nc.declare_dram_parameter("wkvT", [D_MODEL, DKV], F16, isOutput=False)
    wo = nc.declare_dram_parameter("woT", [DQ, D_MODEL], F16, isOutput=False)
    cid = nc.declare_dram_parameter("cid", [P, P + 64], F16, isOutput=False)
    yt0 = nc.declare_dram_parameter("yT0", [D_MODEL, TQ], F16, isOutput=True)
    yt1 = nc.declare_dram_parameter("yT1", [D_MODEL, TQ], F16, isOutput=True)

    with TileContext(nc) as tc:
        with (
            tc.tile_pool(name="consts", bufs=1) as consts,
            tc.tile_pool(name="pt", bufs=3) as ptpool,
            tc.tile_pool(name="nrm", bufs=2) as nrmpool,
            tc.tile_pool(name="yout", bufs=3) as ypool,
            tc.tile_pool(name="psB", bufs=2, space="PSUM") as psB,
            tc.tile_pool(name="psD", bufs=2, space="PSUM") as psD,
        ):
            # ---------------- input staging (full width; loaded in
            # tq-halves per d-tile: each DMA call is 128 rows x 2KB
            # descriptors on its own queue).  xc on the sync queue, xq on
            # the (initially idle) scalar queue, so neither's dispatches
            # stagger the other's transfers.
            xc_t = consts.tile([P, DT, TKV], F16, tag="xc")
            xq_t = consts.tile([P, DT, TQ], F16, tag="xq")
            xc_r = xc.rearrange("(i p) t -> p i t", p=P)
            xq_r = xq.rearrange("(i p) t -> p i t", p=P)
            HALF = TQ // 2
            for i in range(DT):  # wave 1: first halves
                nc.sync.dma_start(xc_t[:, i, :HALF], xc_r[:, i, :HALF])
                nc.scalar.dma_start(xq_t[:, i, :HALF], xq_r[:, i, :HALF])

            ident = consts.tile([P, P + 64], F16, tag="ident")
            nc.sync.dma_start(ident, cid[:])
            wq_sb = consts.tile([P, DT, DQ], F16, tag="wq")
            nc.scalar.dma_start(wq_sb, wq.rearrange("(i p) e -> p i e", p=P))
            wkv_sb = consts.tile([P, DT, DKV], F16, tag="wkv")
            nc.sync.dma_start(wkv_sb, wkv.rearrange("(i p) e -> p i e", p=P))
            wo_sb = consts.tile([P, ET, D_MODEL], F16, tag="wo")
            nc.gpsimd.dma_start(wo_sb, wo.rearrange("(i p) m -> p i m", p=P))

            for i in range(DT):  # wave 2: second halves
                nc.sync.dma_start(xc_t[:, i, HALF:], xc_r[:, i, HALF:])
                nc.scalar.dma_start(xq_t[:, i, HALF:], xq_r[:, i, HALF:])

            qt = consts.tile([P, ET, TQ], F16, tag="qt")  # proj staging
            kv = consts.tile([P, TKV], F16, tag="kv")  # rows 0-63 kT, 64-127 vT
            kc = consts.tile([P, TKV], F16, tag="kc")  # [kT; C-row]
            qt65 = consts.tile([P, HEADS_PER_DEV, TQ], F16, tag="qt65")  # [q_h; 1]
            vp16A = consts.tile([P, NTK, P], F16, tag="vp16A")  # [v | ones]
            vp16B = consts.tile([P, NTK, P], F16, tag="vp16B")  # [ones | v]
            outs = consts.tile([P, ET, TQ], F16, tag="outs")  # normalized outT

            nc.vector.memset(vp16A[:, :, HEAD_DIM:], 1.0)
            nc.vector.memset(vp16B[:, :, :HEAD_DIM], 1.0)
            nc.vector.memset(kc[HEAD_DIM : HEAD_DIM + 1, :], C_DEV)
            nc.vector.memset(qt65[HEAD_DIM : HEAD_DIM + 1, :, :], 1.0)
            bshift = consts.tile([P, 1], F32, tag="bshift")
            nc.vector.memset(bshift, ACT_BIAS)

            # ---------------- PE warmup: ramp the DVFS p-state while the
            # input DMAs land.  Reads an uninitialized SBUF tile (values are
            # irrelevant and the psum is discarded) so it has no dependencies.
            junk = consts.tile([P, FREE], F16, tag="junk")
            nc.gpsimd.memset(junk, 1.0)
            for w in range(10):
                pwarm = psB.tile([P, FREE], F32, tag="ps", name=f"warm{w}")
                for i in range(4):
                    nc.tensor.matmul(
                        pwarm,
                        junk[:, :P],
                        junk[:],
                        start=(i == 0),
                        stop=(i == 3),
                    )

            # ---------------- stage A emitters
            def emit_kv_chunk(c):
                cs = slice(c * FREE, (c + 1) * FREE)
                pkv = psB.tile([P, FREE], F32, tag="ps", name=f"pkv{c}")
                for i in range(DT):
                    nc.tensor.matmul(
                        pkv,
                        wkv_sb[:, i, :],
                        xc_t[:, i, cs],
                        start=(i == 0),
                        stop=(i == DT - 1),
                    )
                nc.vector.tensor_copy(kv[:, cs], pkv)
                h2 = FREE // 2
                for s in range(2):
                    ss = slice(c * FREE + s * h2, c * FREE + (s + 1) * h2)
                    nc.gpsimd.dma_start(kc[:HEAD_DIM, ss], kv[:HEAD_DIM, ss])

            def emit_q_chunk(c, e):
                cs = slice(c * FREE, (c + 1) * FREE)
                pq = psB.tile([P, FREE], F32, tag="ps", name=f"pq{c}_{e}")
                for i in range(DT):
                    nc.tensor.matmul(
                        pq,
                        wq_sb[:, i, e * P : (e + 1) * P],
                        xq_t[:, i, cs],
                        start=(i == 0),
                        stop=(i == DT - 1),
                    )
                nc.vector.tensor_copy(qt[:, e, cs], pq)
                # shift into per-head layout (DMA can cross partitions)
                h2 = FREE // 2
                for s in range(2):
                    ss = slice(c * FREE + s * h2, c * FREE + (s + 1) * h2)
                    nc.gpsimd.dma_start(
                        qt65[:HEAD_DIM, 2 * e, ss], qt[:HEAD_DIM, e, ss]
                    )
                    nc.gpsimd.dma_start(
                        qt65[:HEAD_DIM, 2 * e + 1, ss], qt[HEAD_DIM:, e, ss]
                    )

            def emit_v_tiles(c):
                for t in range(4 * c, 4 * c + 4):
                    ts_ = slice(t * P, (t + 1) * P)
                    pv = psB.tile([P, HEAD_DIM], F16, tag="ps", name=f"pv{t}")
                    nc.tensor.transpose(
                        pv,
                        kv[HEAD_DIM : 2 * HEAD_DIM, ts_],
                        ident[HEAD_DIM:, HEAD_DIM:P],
                    )
                    nc.vector.tensor_copy(vp16A[:, t, :HEAD_DIM], pv)
                    nc.vector.tensor_copy(vp16B[:, t, HEAD_DIM:], pv)

            # ---------------- stage E: one (chunk, m, head-pair) unit
            # writing one of two partial outputs (summed on the host), so
            # each blk's E can start once only its first head-pair is done.
            def emit_out_unit(c, m, ee):
                cs = slice(c * FREE, (c + 1) * FREE)
                ms = slice(m * P, (m + 1) * P)
                ytd = yt0 if ee == 0 else yt1
                py = psB.tile([P, FREE], F32, tag="ps", name=f"py{c}_{m}_{ee}")
                nc.tensor.matmul(py, wo_sb[:, ee, ms], outs[:, ee, cs])
                yo = ypool.tile([P, FREE], F16, tag="yout", name=f"yo{c}_{m}_{ee}")
                nc.vector.tensor_copy(yo, py)
                nc.gpsimd.dma_start(ytd[ms, cs], yo)

            # ---------------- one (blk, head) iteration of B -> exp -> D
            def emit_iteration(blk, e, hpar, extra=None):
                bs = slice(blk * BLK, (blk + 1) * BLK)
                head = 2 * e + hpar
                vp16 = vp16A if hpar == 0 else vp16B

                pd = psD.tile([P, BLK], F32, tag="pd", name=f"pd{blk}_{e}_{hpar}")
                pts = {}

                def emit_d(t, first, last):
                    src = pts.pop(t)
                    for j in range(JPB):
                        jo = slice(j * FREE, (j + 1) * FREE)
                        nc.tensor.matmul(
                            pd[:, jo],
                            vp16[:, t, :],
                            src[:, jo],
                            start=first,
                            stop=last,
                            skip_group_check=True,
                        )

                for t in range(NTK):
                    ts_ = slice(t * P, (t + 1) * P)
                    pb = psB.tile([P, BLK], F32, tag="ps", name=f"pb{t}")
                    for j in range(JPB):
                        js = slice(blk * BLK + j * FREE, blk * BLK + (j + 1) * FREE)
                        jo = slice(j * FREE, (j + 1) * FREE)
                        nc.tensor.matmul(
                            pb[:, jo],
                            kc[: HEAD_DIM + 1, ts_],
                            qt65[: HEAD_DIM + 1, head, js],
                        )
                    pts[t] = ptpool.tile([P, BLK], F16, tag="pt", name=f"pt{t}")
                    nc.scalar.activation(
                        pts[t],
                        pb,
                        mybir.ActivationFunctionType.Exp,
                        bias=bshift[:, :],
                        scale=SCALE,
                    )
                    # D for tile t-2 (software-pipelined behind exp)
                    if t >= 2:
                        emit_d(t - 2, first=(t == 2), last=False)
                    if extra and t in extra:
                        for fn in extra[t]:
                            fn()
                emit_d(NTK - 2, first=False, last=False)
                emit_d(NTK - 1, first=False, last=True)

                # normalize: out = num * approx(1/denom).
                # reciprocal_approx_fast only works at partition base 0, so
                # for even heads (denominator on PSUM rows 64-127) shift the
                # denominator down via SBUF-SBUF DMA first.
                rec = nrmpool.tile([P, BLK], F32, tag="rec")
                if hpar == 0:
                    dnm = nrmpool.tile([P, BLK], F32, tag="dnm")
                    dnm2 = nrmpool.tile([P, BLK], F32, tag="dnm2")
                    nc.vector.tensor_copy(dnm[HEAD_DIM:, :], pd[HEAD_DIM:, :])
                    q4 = HEAD_DIM // 4
                    for s in range(4):
                        nc.sync.dma_start(
                            dnm2[s * q4 : (s + 1) * q4, :],
                            dnm[HEAD_DIM + s * q4 : HEAD_DIM + (s + 1) * q4, :],
                        )
                    nc.vector.reciprocal_approx_fast(
                        rec[:HEAD_DIM, :], dnm2[:HEAD_DIM, :]
                    )
                    nc.vector.tensor_mul(
                        outs[:HEAD_DIM, e, bs], pd[:HEAD_DIM, :], rec[:HEAD_DIM, :]
                    )
                else:
                    rec2 = nrmpool.tile([P, BLK], F32, tag="rec2")
                    nc.vector.reciprocal_approx_fast(rec[:HEAD_DIM, :], pd[:HEAD_DIM, :])
                    q4 = HEAD_DIM // 4
                    for s in range(4):
                        nc.sync.dma_start(
                            rec2[HEAD_DIM + s * q4 : HEAD_DIM + (s + 1) * q4, :],
                            rec[s * q4 : (s + 1) * q4, :],
                        )
                    nc.vector.tensor_mul(
                        outs[HEAD_DIM:, e, bs], pd[HEAD_DIM:, :], rec2[HEAD_DIM:, :]
                    )

            # ---------------- emission schedule
            emit_kv_chunk(0)
            emit_kv_chunk(1)
            emit_q_chunk(0, 0)
            emit_q_chunk(1, 0)
            emit_v_tiles(0)
            emit_v_tiles(1)

            def eu(c, m, ee):
                return lambda: emit_out_unit(c, m, ee)

            def merge(*dicts):
                out = {}
                for d in dicts:
                    for k, v in d.items():
                        out.setdefault(k, []).append(v)
                return out

            emit_iteration(
                0, 0, 0,
                extra=merge({
                    2: lambda: emit_q_chunk(0, 1),
                    4: lambda: emit_q_chunk(1, 1),
                    6: lambda: emit_kv_chunk(2),
                    7: lambda: emit_v_tiles(2),
                    8: lambda: emit_kv_chunk(3),
                    9: lambda: emit_v_tiles(3),
                }),
            )
            emit_iteration(
                0, 0, 1,
                extra=merge(
                    {3: lambda: emit_q_chunk(2, 0), 6: lambda: emit_q_chunk(3, 0)}
                ),
            )
            # E for (blk, head-pair) hides inside the two following iterations
            emit_iteration(
                0, 1, 0,
                extra=merge(
                    {4: lambda: emit_q_chunk(2, 1), 6: lambda: emit_q_chunk(3, 1)},
                    {2 * m + 1: eu(0, m, 0) for m in range(MT)},
                ),
            )
            emit_iteration(
                0, 1, 1, extra=merge({2 * m + 1: eu(1, m, 0) for m in range(MT)})
            )
            emit_iteration(
                1, 0, 0, extra=merge({2 * m + 1: eu(0, m, 1) for m in range(MT)})
            )
            emit_iteration(
                1, 0, 1, extra=merge({2 * m + 1: eu(1, m, 1) for m in range(MT)})
            )
            emit_iteration(
                1, 1, 0, extra=merge({2 * m + 1: eu(2, m, 0) for m in range(MT)})
            )
            emit_iteration(
                1, 1, 1, extra=merge({2 * m + 1: eu(3, m, 0) for m in range(MT)})
            )
            for c in (2, 3):
                for m in range(MT):
                    emit_out_unit(c, m, 1)

    nc.finalize()
    return nc


_NC_CACHE = None


def _get_nc():
    global _NC_CACHE
    if _NC_CACHE is None:
        _NC_CACHE = build_bass()
    return _NC_CACHE


def _cid():
    c = np.zeros((P, P + 64), dtype=np.float16)
    c[:, :P] = np.eye(P, dtype=np.float32)
    c[:, P:] = 1.0
    return c


def shard_inputs(query, context, Wq, Wk, Wv, Wo):
    """host-side sharding: 8 cores = batch(2) x kv-group(4)"""
    in_maps = []
    xqT = [np.ascontiguousarray(query[b].T).astype(np.float16) for b in range(B)]
    xcT = [np.ascontiguousarray(context[b].T).astype(np.float16) for b in range(B)]
    for core in range(N_CORES):
        b, g = divmod(core, GROUPS)
        wqT = np.ascontiguousarray(Wq[g * DQ : (g + 1) * DQ, :].T).astype(np.float16)
        wkvT = np.ascontiguousarray(
            np.concatenate(
                [
                    Wk[g * HEAD_DIM : (g + 1) * HEAD_DIM, :],
                    Wv[g * HEAD_DIM : (g + 1) * HEAD_DIM, :],
                ],
                axis=0,
            ).T
        ).astype(np.float16)
        woT = np.ascontiguousarray(Wo[:, g * DQ : (g + 1) * DQ].T).astype(np.float16)
        in_maps.append(
            {
                "xqT": xqT[b],
                "xcT": xcT[b],
                "wqT": wqT,
                "wkvT": wkvT,
                "woT": woT,
                "cid": _cid(),
            }
        )
    return in_maps


def kernel(query, context, Wq, Wk, Wv, Wo, _want_profile=False):
    from concourse.bass_utils import run_bass_kernel_spmd

    nc = _get_nc()
    in_maps = shard_inputs(query, context, Wq, Wk, Wv, Wo)
    res = run_bass_kernel_spmd(
        nc, in_maps, core_ids=list(range(N_CORES)), trace=_want_profile
    )
    out = np.zeros((B, TQ, D_MODEL), dtype=np.float32)
    for core in range(N_CORES):
        b = core // GROUPS
        out[b] += res.results[core]["yT0"].T.astype(np.float32)
        out[b] += res.results[core]["yT1"].T.astype(np.float32)
    if _want_profile:
        return out, res
    return out


# revision 37
# speedup vs baseline: 1.0399x; 1.0399x over previous
"""Cross-attention (GQA) Trainium2 Bass kernel, v3.

Problem: B=2, Tq=Tkv=2048, D_MODEL=1024, 16 query heads / 4 kv heads,
head_dim=64.  Sharded over 8 NeuronCores as batch(2) x kv-group(4); each
core computes 4 query heads + its single kv head and a partial output
projection (Wo row-split by head group); partials are summed on host.

Design (v3):
  * single-head (blk x head) iterations with a pipelined B->exp->D
    schedule; exp is true f16 exp on ACT (the DVE bit-trick exp and fp8
    P@V measured 1.8-2.9e-2 max-rel-err -- peaked softmax columns expose
    P's relative quantization directly -- vs the 2e-2 gate, so they're
    disabled).  Steady state is ACT-paced at ~1.0us/tile.
  * scores carry a constant offset row (stationary row 64 = C, moving
    row 64 = 1) kept from the bit-exp experiments; ACT's free bias
    compensates.  Numerically neutral, it keeps exp outputs in a safe
    f16 range for any score outliers.
  * softmax denominators from ones-columns in the P@V stationary;
    reciprocal_approx_fast (only on partitions 0-63: its custom-DVE
    uops are broken at nonzero partition offsets).
  * input DMAs use full-row 2KB descriptors (descriptor count, not
    bytes, bounds DMA: ~57ns/descriptor/queue), split in tq-halves
    across queues; DMA dispatches are spread over the sync, gpsimd and
    vector sequencers (each DIRECT2D dispatch costs ~0.6us serialized
    on its issuing engine's sequencer).
  * E (output projection) for blk0 is interleaved into blk1's
    iterations; output yT is f16 (host accumulates in f32).
  * PSUM: scores pool 2x2 banks, accumulator pool 2x2 banks; PE warmup
    ramps the DVFS p-state (1.2 -> 2.4 GHz) before the first projection.
"""

import os
import sys

import numpy as np

for _p in ("/opt/trn_rl_repo",):
    if _p not in sys.path and os.path.isdir(_p):
        sys.path.insert(0, _p)

import concourse.bass as bass
import concourse.bacc as bacc
import concourse.mybir as mybir
from concourse.tile import TileContext

# ---------------------------------------------------------------- problem dims
B = 2
TQ = 2048
TKV = 2048
D_MODEL = 1024
N_HEADS = 16
N_KV_HEADS = 4
HEAD_DIM = 64
N_CORES = 8
GROUPS = N_KV_HEADS  # kv groups = 4
HEADS_PER_DEV = N_HEADS // GROUPS  # 4
DQ = HEADS_PER_DEV * HEAD_DIM  # 256
DKV = 2 * HEAD_DIM  # 128 (k rows + v rows stacked)
SCALE = 1.0 / float(np.sqrt(HEAD_DIM))

P = 128
FREE = 512  # matmul moving-operand chunk / psum bank width
BLK = 1024  # tq block width

F32 = mybir.dt.float32
F16 = mybir.dt.float16

DT = D_MODEL // P  # 8 d-tiles
ET = DQ // P  # 2 e-tiles (query head pairs)
NCH = TQ // FREE  # 4 chunks of 512
NTK = TKV // P  # 16 tk tiles
NBLK = TQ // BLK  # 2 tq blocks
JPB = BLK // FREE  # 2 free-chunks per block
MT = D_MODEL // P  # 8 output m-tiles

# exp scale handling: scores carry +C_DEV via an offset row; ACT's bias
# restores e^(s/8 - ln2*(15 - AEXP*C/1024)) -- a benign global scale that
# cancels in the softmax normalization.
SHIFT = 3.75
AEXP = 1024.0 * 1.4426950408889634 / 8.0
_LOG2E = 1.4426950408889634
C_OFFSET = (15360.0 - 44.5 - SHIFT * 1024.0 * _LOG2E) / AEXP
C_DEV = float(np.float16(C_OFFSET))
ACT_BIAS = float(np.log(2.0) * (AEXP * C_DEV / 1024.0 - 15.0) - C_DEV / 8.0)


def build_bass():
    nc = bacc.Bacc()

    xq = nc.declare_dram_parameter("xqT", [D_MODEL, TQ], F16, isOutput=False)
    xc = nc.declare_dram_parameter("xcT", [D_MODEL, TKV], F16, isOutput=False)
    wq = nc.declare_dram_parameter("wqT", [D_MODEL, DQ], F16, isOutput=False)
    wkv = nc.declare_dram_parameter("wkvT", [D_MODEL, DKV], F16, isOutput=False)
    wo = nc.declare_dram_parameter("woT", [DQ, D_MODEL], F16, isOutput=False)
    cid = nc.declare_dram_parameter("cid", [P, P + 64], F16, isOutput=False)
    yt0 = nc.declare_dram_parameter("yT0", [D_MODEL, TQ], F16, isOutput=True)
    yt1 = nc.declare_dram_parameter("yT1", [D_MODEL, TQ], F16, isOutput=True)

    with TileContext(nc) as tc:
        with (
            tc.tile_pool(name="consts", bufs=1) as consts,
            tc.tile_pool(name="pt", bufs=3) as ptpool,
            tc.tile_pool(name="nrm", bufs=2) as nrmpool,
            tc.tile_pool(name="yout", bufs=3) as ypool,
            tc.tile_pool(name="psb1", bufs=2, space="PSUM") as psb1,
            tc.tile_pool(name="psb2", bufs=2, space="PSUM") as psb2,
            tc.tile_pool(name="psD", bufs=1, space="PSUM") as psD,
        ):
            # ---------------- input staging (full width; loaded in
            # tq-halves per d-tile: each DMA call is 128 rows x 2KB
            # descriptors on its own queue).  xc on the sync queue, xq on
            # the (initially idle) scalar queue, so neither's dispatches
            # stagger the other's transfers.
            xc_t = consts.tile([P, DT, TKV], F16, tag="xc")
            xq_t = consts.tile([P, DT, TQ], F16, tag="xq")
            xc_r = xc.rearrange("(i p) t -> p i t", p=P)
            xq_r = xq.rearrange("(i p) t -> p i t", p=P)
            HALF = TQ // 2
            for i in range(DT):  # wave 1: first halves
                nc.sync.dma_start(xc_t[:, i, :HALF], xc_r[:, i, :HALF])
                nc.scalar.dma_start(xq_t[:, i, :HALF], xq_r[:, i, :HALF])

            ident = consts.tile([P, P + 64], F16, tag="ident")
            nc.sync.dma_start(ident, cid[:])
            wq_sb = consts.tile([P, DT, DQ], F16, tag="wq")
            nc.scalar.dma_start(wq_sb, wq.rearrange("(i p) e -> p i e", p=P))
            wkv_sb = consts.tile([P, DT, DKV], F16, tag="wkv")
            nc.sync.dma_start(wkv_sb, wkv.rearrange("(i p) e -> p i e", p=P))
            wo_sb = consts.tile([P, ET, D_MODEL], F16, tag="wo")
            nc.gpsimd.dma_start(wo_sb, wo.rearrange("(i p) m -> p i m", p=P))

            for i in range(DT):  # wave 2: second halves
                nc.sync.dma_start(xc_t[:, i, HALF:], xc_r[:, i, HALF:])
                nc.scalar.dma_start(xq_t[:, i, HALF:], xq_r[:, i, HALF:])

            qt = consts.tile([P, ET, TQ], F16, tag="qt")  # proj staging
            kv = consts.tile([P, TKV], F16, tag="kv")  # rows 0-63 kT, 64-127 vT
            kc = consts.tile([P, TKV], F16, tag="kc")  # [kT; C-row]
            qt65 = consts.tile([P, HEADS_PER_DEV, TQ], F16, tag="qt65")  # [q_h; 1]
            vp16A = consts.tile([P, NTK, P], F16, tag="vp16A")  # [v | ones]
            vp16B = consts.tile([P, NTK, P], F16, tag="vp16B")  # [ones | v]
            outs = consts.tile([P, ET, TQ], F16, tag="outs")  # normalized outT

            nc.vector.memset(vp16A[:, :, HEAD_DIM:], 1.0)
            nc.vector.memset(vp16B[:, :, :HEAD_DIM], 1.0)
            nc.vector.memset(kc[HEAD_DIM : HEAD_DIM + 1, :], C_DEV)
            nc.vector.memset(qt65[HEAD_DIM : HEAD_DIM + 1, :, :], 1.0)
            bshift = consts.tile([P, 1], F32, tag="bshift")
            nc.vector.memset(bshift, ACT_BIAS)

            # ---------------- PE warmup: ramp the DVFS p-state while the
            # input DMAs land.  Reads an uninitialized SBUF tile (values are
            # irrelevant and the psum is discarded) so it has no dependencies.
            junk = consts.tile([P, FREE], F16, tag="junk")
            nc.gpsimd.memset(junk, 1.0)
            for w in range(10):
                pwarm = psb2.tile([P, FREE], F32, tag="ps2", name=f"warm{w}")
                for i in range(4):
                    nc.tensor.matmul(
                        pwarm,
                        junk[:, :P],
                        junk[:],
                        start=(i == 0),
                        stop=(i == 3),
                    )

            # ---------------- stage A emitters
            def emit_kv_chunk(c):
                cs = slice(c * FREE, (c + 1) * FREE)
                pkv = psb2.tile([P, FREE], F32, tag="ps2", name=f"pkv{c}")
                for i in range(DT):
                    nc.tensor.matmul(
                        pkv,
                        wkv_sb[:, i, :],
                        xc_t[:, i, cs],
                        start=(i == 0),
                        stop=(i == DT - 1),
                    )
                nc.vector.tensor_copy(kv[:, cs], pkv)
                h2 = FREE // 2
                for s in range(2):
                    ss = slice(c * FREE + s * h2, c * FREE + (s + 1) * h2)
                    nc.gpsimd.dma_start(kc[:HEAD_DIM, ss], kv[:HEAD_DIM, ss])

            def emit_q_chunk(c, e):
                cs = slice(c * FREE, (c + 1) * FREE)
                pq = psb2.tile([P, FREE], F32, tag="ps2", name=f"pq{c}_{e}")
                for i in range(DT):
                    nc.tensor.matmul(
                        pq,
                        wq_sb[:, i, e * P : (e + 1) * P],
                        xq_t[:, i, cs],
                        start=(i == 0),
                        stop=(i == DT - 1),
                    )
                nc.vector.tensor_copy(qt[:, e, cs], pq)
                # shift into per-head layout (DMA can cross partitions)
                h2 = FREE // 2
                for s in range(2):
                    ss = slice(c * FREE + s * h2, c * FREE + (s + 1) * h2)
                    nc.gpsimd.dma_start(
                        qt65[:HEAD_DIM, 2 * e, ss], qt[:HEAD_DIM, e, ss]
                    )
                    nc.gpsimd.dma_start(
                        qt65[:HEAD_DIM, 2 * e + 1, ss], qt[HEAD_DIM:, e, ss]
                    )

            def emit_v_tiles(c):
                for t in range(4 * c, 4 * c + 4):
                    ts_ = slice(t * P, (t + 1) * P)
                    pv = psb2.tile([P, HEAD_DIM], F16, tag="ps2", name=f"pv{t}")
                    nc.tensor.transpose(
                        pv,
                        kv[HEAD_DIM : 2 * HEAD_DIM, ts_],
                        ident[HEAD_DIM:, HEAD_DIM:P],
                    )
                    nc.vector.tensor_copy(vp16A[:, t, :HEAD_DIM], pv)
                    nc.vector.tensor_copy(vp16B[:, t, HEAD_DIM:], pv)

            # ---------------- stage E: one (chunk, m, head-pair) unit
            # writing one of two partial outputs (summed on the host), so
            # each blk's E can start once only its first head-pair is done.
            def emit_out_unit(c, m, ee):
                cs = slice(c * FREE, (c + 1) * FREE)
                ms = slice(m * P, (m + 1) * P)
                ytd = yt0 if ee == 0 else yt1
                py = psb2.tile([P, FREE], F32, tag="ps2", name=f"py{c}_{m}_{ee}")
                nc.tensor.matmul(py, wo_sb[:, ee, ms], outs[:, ee, cs])
                yo = ypool.tile([P, FREE], F16, tag="yout", name=f"yo{c}_{m}_{ee}")
                nc.vector.tensor_copy(yo, py)
                nc.gpsimd.dma_start(ytd[ms, cs], yo)

            # ---------------- one (blk, head) iteration of B -> exp -> D
            def emit_iteration(blk, e, hpar, extra=None):
                bs = slice(blk * BLK, (blk + 1) * BLK)
                head = 2 * e + hpar
                vp16 = vp16A if hpar == 0 else vp16B

                pd = psD.tile([P, BLK], F32, tag="pd", name=f"pd{blk}_{e}_{hpar}")
                pts = {}

                def emit_d(t, first, last):
                    src = pts.pop(t)
                    for j in range(JPB):
                        jo = slice(j * FREE, (j + 1) * FREE)
                        nc.tensor.matmul(
                            pd[:, jo],
                            vp16[:, t, :],
                            src[:, jo],
                            start=first,
                            stop=last,
                            skip_group_check=True,
                        )

                for t in range(NTK):
                    ts_ = slice(t * P, (t + 1) * P)
                    pb = psb1.tile([P, BLK], F32, tag="ps1", name=f"pb{t}")
                    for j in range(JPB):
                        js = slice(blk * BLK + j * FREE, blk * BLK + (j + 1) * FREE)
                        jo = slice(j * FREE, (j + 1) * FREE)
                        nc.tensor.matmul(
                            pb[:, jo],
                            kc[: HEAD_DIM + 1, ts_],
                            qt65[: HEAD_DIM + 1, head, js],
                        )
                    pts[t] = ptpool.tile([P, BLK], F16, tag="pt", name=f"pt{t}")
                    nc.scalar.activation(
                        pts[t],
                        pb,
                        mybir.ActivationFunctionType.Exp,
                        bias=bshift[:, :],
                        scale=SCALE,
                    )
                    # D for tile t-2 (software-pipelined behind exp)
                    if t >= 2:
                        emit_d(t - 2, first=(t == 2), last=False)
                    if extra and t in extra:
                        for fn in extra[t]:
                            fn()
                emit_d(NTK - 2, first=False, last=False)
                emit_d(NTK - 1, first=False, last=True)

                # normalize: out = num * approx(1/denom).
                # reciprocal_approx_fast only works at partition base 0, so
                # for even heads (denominator on PSUM rows 64-127) shift the
                # denominator down via SBUF-SBUF DMA first.
                rec = nrmpool.tile([P, BLK], F32, tag="rec")
                raw = nrmpool.tile([P, BLK], F32, tag="raw")
                q4 = HEAD_DIM // 4
                if hpar == 0:
                    dnm = nrmpool.tile([P, BLK], F32, tag="dnm")
                    dnm2 = nrmpool.tile([P, BLK], F32, tag="dnm2")
                    nc.vector.tensor_copy(raw[:HEAD_DIM, :], pd[:HEAD_DIM, :])
                    nc.vector.tensor_copy(dnm[HEAD_DIM:, :], pd[HEAD_DIM:, :])
                    for s in range(4):
                        nc.sync.dma_start(
                            dnm2[s * q4 : (s + 1) * q4, :],
                            dnm[HEAD_DIM + s * q4 : HEAD_DIM + (s + 1) * q4, :],
                        )
                    nc.vector.reciprocal_approx_fast(
                        rec[:HEAD_DIM, :], dnm2[:HEAD_DIM, :]
                    )
                    nc.vector.tensor_mul(
                        outs[:HEAD_DIM, e, bs], raw[:HEAD_DIM, :], rec[:HEAD_DIM, :]
                    )
                else:
                    rec2 = nrmpool.tile([P, BLK], F32, tag="rec2")
                    nc.vector.reciprocal_approx_fast(rec[:HEAD_DIM, :], pd[:HEAD_DIM, :])
                    nc.vector.tensor_copy(raw[HEAD_DIM:, :], pd[HEAD_DIM:, :])
                    for s in range(4):
                        nc.sync.dma_start(
                            rec2[HEAD_DIM + s * q4 : HEAD_DIM + (s + 1) * q4, :],
                            rec[s * q4 : (s + 1) * q4, :],
                        )
                    nc.vector.tensor_mul(
                        outs[HEAD_DIM:, e, bs], raw[HEAD_DIM:, :], rec2[HEAD_DIM:, :]
                    )

            # ---------------- emission schedule
            emit_kv_chunk(0)
            emit_kv_chunk(1)
            emit_q_chunk(0, 0)
            emit_q_chunk(1, 0)
            emit_v_tiles(0)
            emit_v_tiles(1)

            def eu(c, m, ee):
                return lambda: emit_out_unit(c, m, ee)

            def merge(*dicts):
                out = {}
                for d in dicts:
                    for k, v in d.items():
                        out.setdefault(k, []).append(v)
                return out

            emit_iteration(
                0, 0, 0,
                extra=merge({
                    2: lambda: emit_q_chunk(0, 1),
                    4: lambda: emit_q_chunk(1, 1),
                    6: lambda: emit_kv_chunk(2),
                    7: lambda: emit_v_tiles(2),
                    8: lambda: emit_kv_chunk(3),
                    9: lambda: emit_v_tiles(3),
                }),
            )
            emit_iteration(
                0, 0, 1,
                extra=merge(
                    {3: lambda: emit_q_chunk(2, 0), 6: lambda: emit_q_chunk(3, 0)}
                ),
            )
            # E for (blk, head-pair) hides inside the two following iterations
            emit_iteration(
                0, 1, 0,
                extra=merge(
                    {4: lambda: emit_q_chunk(2, 1), 6: lambda: emit_q_chunk(3, 1)},
                    {2 * m + 1: eu(0, m, 0) for m in range(MT)},
                ),
            )
            emit_iteration(
                0, 1, 1, extra=merge({2 * m + 1: eu(1, m, 0) for m in range(MT)})
            )
            emit_iteration(
                1, 0, 0, extra=merge({2 * m + 1: eu(0, m, 1) for m in range(MT)})
            )
            emit_iteration(
                1, 0, 1, extra=merge({2 * m + 1: eu(1, m, 1) for m in range(MT)})
            )
            emit_iteration(
                1, 1, 0, extra=merge({2 * m + 1: eu(2, m, 0) for m in range(MT)})
            )
            emit_iteration(
                1, 1, 1, extra=merge({2 * m + 1: eu(3, m, 0) for m in range(MT)})
            )
            for c in (2, 3):
                for m in range(MT):
                    emit_out_unit(c, m, 1)

    nc.finalize()
    return nc


_NC_CACHE = None


def _get_nc():
    global _NC_CACHE
    if _NC_CACHE is None:
        _NC_CACHE = build_bass()
    return _NC_CACHE


def _cid():
    c = np.zeros((P, P + 64), dtype=np.float16)
    c[:, :P] = np.eye(P, dtype=np.float32)
    c[:, P:] = 1.0
    return c


def shard_inputs(query, context, Wq, Wk, Wv, Wo):
    """host-side sharding: 8 cores = batch(2) x kv-group(4)"""
    in_maps = []
    xqT = [np.ascontiguousarray(query[b].T).astype(np.float16) for b in range(B)]
    xcT = [np.ascontiguousarray(context[b].T).astype(np.float16) for b in range(B)]
    for core in range(N_CORES):
        b, g = divmod(core, GROUPS)
        wqT = np.ascontiguousarray(Wq[g * DQ : (g + 1) * DQ, :].T).astype(np.float16)
        wkvT = np.ascontiguousarray(
            np.concatenate(
                [
                    Wk[g * HEAD_DIM : (g + 1) * HEAD_DIM, :],
                    Wv[g * HEAD_DIM : (g + 1) * HEAD_DIM, :],
                ],
                axis=0,
            ).T
        ).astype(np.float16)
        woT = np.ascontiguousarray(Wo[:, g * DQ : (g + 1) * DQ].T).astype(np.float16)
        in_maps.append(
            {
                "xqT": xqT[b],
                "xcT": xcT[b],
                "wqT": wqT,
                "wkvT": wkvT,
                "woT": woT,
                "cid": _cid(),
            }
        )
    return in_maps


def kernel(query, context, Wq, Wk, Wv, Wo, _want_profile=False):
    from concourse.bass_utils import run_bass_kernel_spmd

    nc = _get_nc()
    in_maps = shard_inputs(query, context, Wq, Wk, Wv, Wo)
    res = run_bass_kernel_spmd(
        nc, in_maps, core_ids=list(range(N_CORES)), trace=_want_profile
    )
    out = np.zeros((B, TQ, D_MODEL), dtype=np.float32)
    for core in range(N_CORES):
        b = core // GROUPS
        out[b] += res.results[core]["yT0"].T.astype(np.float32)
        out[b] += res.results[core]["yT1"].T.astype(np.float32)
    if _want_profile:
        return out, res
    return out


# revision 38
# speedup vs baseline: 1.1753x; 1.1303x over previous
"""Cross-attention (GQA) Trainium2 Bass kernel, v3.

Problem: B=2, Tq=Tkv=2048, D_MODEL=1024, 16 query heads / 4 kv heads,
head_dim=64.  Sharded over 8 NeuronCores as batch(2) x kv-group(4); each
core computes 4 query heads + its single kv head and a partial output
projection (Wo row-split by head group); partials are summed on host.

Design (v3):
  * single-head (blk x head) iterations with a pipelined B->exp->D
    schedule; exp is true f16 exp on ACT (the DVE bit-trick exp and fp8
    P@V measured 1.8-2.9e-2 max-rel-err -- peaked softmax columns expose
    P's relative quantization directly -- vs the 2e-2 gate, so they're
    disabled).  Steady state is ACT-paced at ~1.0us/tile.
  * scores carry a constant offset row (stationary row 64 = C, moving
    row 64 = 1) kept from the bit-exp experiments; ACT's free bias
    compensates.  Numerically neutral, it keeps exp outputs in a safe
    f16 range for any score outliers.
  * softmax denominators from ones-columns in the P@V stationary;
    reciprocal_approx_fast (only on partitions 0-63: its custom-DVE
    uops are broken at nonzero partition offsets).
  * input DMAs use full-row 2KB descriptors (descriptor count, not
    bytes, bounds DMA: ~57ns/descriptor/queue), split in tq-halves
    across queues; DMA dispatches are spread over the sync, gpsimd and
    vector sequencers (each DIRECT2D dispatch costs ~0.6us serialized
    on its issuing engine's sequencer).
  * E (output projection) for blk0 is interleaved into blk1's
    iterations; output yT is f16 (host accumulates in f32).
  * PSUM: scores pool 2x2 banks, accumulator pool 2x2 banks; PE warmup
    ramps the DVFS p-state (1.2 -> 2.4 GHz) before the first projection.
"""

import os
import sys

import numpy as np

for _p in ("/opt/trn_rl_repo",):
    if _p not in sys.path and os.path.isdir(_p):
        sys.path.insert(0, _p)

import concourse.bass as bass
import concourse.bacc as bacc
import concourse.mybir as mybir
from concourse.tile import TileContext

# ---------------------------------------------------------------- problem dims
B = 2
TQ = 2048
TKV = 2048
D_MODEL = 1024
N_HEADS = 16
N_KV_HEADS = 4
HEAD_DIM = 64
N_CORES = 8
GROUPS = N_KV_HEADS  # kv groups = 4
HEADS_PER_DEV = N_HEADS // GROUPS  # 4
DQ = HEADS_PER_DEV * HEAD_DIM  # 256
DKV = 2 * HEAD_DIM  # 128 (k rows + v rows stacked)
SCALE = 1.0 / float(np.sqrt(HEAD_DIM))

P = 128
FREE = 512  # matmul moving-operand chunk / psum bank width
BLK = 1024  # tq block width

F32 = mybir.dt.float32
F16 = mybir.dt.float16

DT = D_MODEL // P  # 8 d-tiles
ET = DQ // P  # 2 e-tiles (query head pairs)
NCH = TQ // FREE  # 4 chunks of 512
NTK = TKV // P  # 16 tk tiles
NBLK = TQ // BLK  # 2 tq blocks
JPB = BLK // FREE  # 2 free-chunks per block
MT = D_MODEL // P  # 8 output m-tiles

# exp scale handling: scores carry +C_DEV via an offset row; ACT's bias
# restores e^(s/8 - ln2*(15 - AEXP*C/1024)) -- a benign global scale that
# cancels in the softmax normalization.
SHIFT = 3.75
AEXP = 1024.0 * 1.4426950408889634 / 8.0
_LOG2E = 1.4426950408889634
C_OFFSET = (15360.0 - 44.5 - SHIFT * 1024.0 * _LOG2E) / AEXP
C_DEV = float(np.float16(C_OFFSET))
ACT_BIAS = float(np.log(2.0) * (AEXP * C_DEV / 1024.0 - 15.0) - C_DEV / 8.0)


def build_bass():
    nc = bacc.Bacc()

    xq = nc.declare_dram_parameter("xqT", [D_MODEL, TQ], F16, isOutput=False)
    xc = nc.declare_dram_parameter("xcT", [D_MODEL, TKV], F16, isOutput=False)
    wq = nc.declare_dram_parameter("wqT", [D_MODEL, DQ], F16, isOutput=False)
    wkv = nc.declare_dram_parameter("wkvT", [D_MODEL, DKV], F16, isOutput=False)
    wo = nc.declare_dram_parameter("woT", [DQ, D_MODEL], F16, isOutput=False)
    cid = nc.declare_dram_parameter("cid", [P, P + 64], F16, isOutput=False)
    yt = nc.declare_dram_parameter("yT", [D_MODEL, TQ], F16, isOutput=True)

    with TileContext(nc) as tc:
        with (
            tc.tile_pool(name="consts", bufs=1) as consts,
            tc.tile_pool(name="pt", bufs=3) as ptpool,
            tc.tile_pool(name="nrm", bufs=2) as nrmpool,
            tc.tile_pool(name="yout", bufs=3) as ypool,
            tc.tile_pool(name="psB", bufs=2, space="PSUM") as psB,
            tc.tile_pool(name="psD", bufs=2, space="PSUM") as psD,
        ):
            # ---------------- input staging (full width; tq-halves per
            # d-tile: each call is 128 rows x 2KB descriptors on one queue).
            # xc dispatches on sync, xq on the initially-idle scalar queue.
            xc_t = consts.tile([P, DT, TKV], F16, tag="xc")
            xq_t = consts.tile([P, DT, TQ], F16, tag="xq")
            xc_r = xc.rearrange("(i p) t -> p i t", p=P)
            xq_r = xq.rearrange("(i p) t -> p i t", p=P)
            HALF = TQ // 2
            for i in range(DT):  # wave 1: first halves
                nc.sync.dma_start(xc_t[:, i, :HALF], xc_r[:, i, :HALF])
                nc.scalar.dma_start(xq_t[:, i, :HALF], xq_r[:, i, :HALF])

            ident = consts.tile([P, P + 64], F16, tag="ident")
            nc.sync.dma_start(ident, cid[:])
            wq_sb = consts.tile([P, DT, DQ], F16, tag="wq")
            nc.scalar.dma_start(wq_sb, wq.rearrange("(i p) e -> p i e", p=P))
            wkv_sb = consts.tile([P, DT, DKV], F16, tag="wkv")
            nc.sync.dma_start(wkv_sb, wkv.rearrange("(i p) e -> p i e", p=P))
            wo_sb = consts.tile([P, ET, D_MODEL], F16, tag="wo")
            nc.gpsimd.dma_start(wo_sb, wo.rearrange("(i p) m -> p i m", p=P))

            for i in range(DT):  # wave 2: second halves
                nc.sync.dma_start(xc_t[:, i, HALF:], xc_r[:, i, HALF:])
                nc.scalar.dma_start(xq_t[:, i, HALF:], xq_r[:, i, HALF:])

            qt = consts.tile([P, ET, TQ], F16, tag="qt")  # proj staging
            kv = consts.tile([P, TKV], F16, tag="kv")  # rows 0-63 kT, 64-127 vT
            kc = consts.tile([P, TKV], F16, tag="kc")  # [kT; C-row]
            qt65 = consts.tile([P, HEADS_PER_DEV, TQ], F16, tag="qt65")  # [q_h; 1]
            vp16A = consts.tile([P, NTK, P], F16, tag="vp16A")  # [v | ones]
            vp16B = consts.tile([P, NTK, P], F16, tag="vp16B")  # [ones | v]
            outs = consts.tile([P, ET, TQ], F16, tag="outs")  # normalized outT

            nc.vector.memset(vp16A[:, :, HEAD_DIM:], 1.0)
            nc.vector.memset(vp16B[:, :, :HEAD_DIM], 1.0)
            nc.vector.memset(kc[HEAD_DIM : HEAD_DIM + 1, :], C_DEV)
            nc.vector.memset(qt65[HEAD_DIM : HEAD_DIM + 1, :, :], 1.0)
            bshift = consts.tile([P, 1], F32, tag="bshift")
            nc.vector.memset(bshift, ACT_BIAS)

            # ---------------- PE warmup: ramp the DVFS p-state while the
            # input DMAs land (reads a memset junk tile: no DMA dependency).
            junk = consts.tile([P, FREE], F16, tag="junk")
            nc.gpsimd.memset(junk, 1.0)
            for w in range(10):
                pwarm = psB.tile([P, FREE], F32, tag="ps", name=f"warm{w}")
                for i in range(4):
                    nc.tensor.matmul(
                        pwarm,
                        junk[:, :P],
                        junk[:],
                        start=(i == 0),
                        stop=(i == 3),
                    )

            # ---------------- stage A emitters
            def emit_kv_chunk(c):
                cs = slice(c * FREE, (c + 1) * FREE)
                pkv = psB.tile([P, FREE], F32, tag="ps", name=f"pkv{c}")
                for i in range(DT):
                    nc.tensor.matmul(
                        pkv,
                        wkv_sb[:, i, :],
                        xc_t[:, i, cs],
                        start=(i == 0),
                        stop=(i == DT - 1),
                    )
                nc.vector.tensor_copy(kv[:, cs], pkv)
                h2 = FREE // 2
                for s in range(2):
                    ss = slice(c * FREE + s * h2, c * FREE + (s + 1) * h2)
                    nc.gpsimd.dma_start(kc[:HEAD_DIM, ss], kv[:HEAD_DIM, ss])

            def emit_q_chunk(c, e):
                cs = slice(c * FREE, (c + 1) * FREE)
                pq = psB.tile([P, FREE], F32, tag="ps", name=f"pq{c}_{e}")
                for i in range(DT):
                    nc.tensor.matmul(
                        pq,
                        wq_sb[:, i, e * P : (e + 1) * P],
                        xq_t[:, i, cs],
                        start=(i == 0),
                        stop=(i == DT - 1),
                    )
                nc.vector.tensor_copy(qt[:, e, cs], pq)
                # shift into per-head layout (DMA can cross partitions)
                h2 = FREE // 2
                for s in range(2):
                    ss = slice(c * FREE + s * h2, c * FREE + (s + 1) * h2)
                    nc.gpsimd.dma_start(
                        qt65[:HEAD_DIM, 2 * e, ss], qt[:HEAD_DIM, e, ss]
                    )
                    nc.gpsimd.dma_start(
                        qt65[:HEAD_DIM, 2 * e + 1, ss], qt[HEAD_DIM:, e, ss]
                    )

            def emit_v_tiles(c):
                for t in range(4 * c, 4 * c + 4):
                    ts_ = slice(t * P, (t + 1) * P)
                    pv = psB.tile([P, HEAD_DIM], F16, tag="ps", name=f"pv{t}")
                    nc.tensor.transpose(
                        pv,
                        kv[HEAD_DIM : 2 * HEAD_DIM, ts_],
                        ident[HEAD_DIM:, HEAD_DIM:P],
                    )
                    nc.vector.tensor_copy(vp16A[:, t, :HEAD_DIM], pv)
                    nc.vector.tensor_copy(vp16B[:, t, HEAD_DIM:], pv)

            # ---------------- stage E: one (chunk, m) unit
            def emit_out_unit(c, m):
                cs = slice(c * FREE, (c + 1) * FREE)
                ms = slice(m * P, (m + 1) * P)
                py = psB.tile([P, FREE], F32, tag="ps", name=f"py{c}_{m}")
                for ee in range(ET):
                    nc.tensor.matmul(
                        py,
                        wo_sb[:, ee, ms],
                        outs[:, ee, cs],
                        start=(ee == 0),
                        stop=(ee == ET - 1),
                    )
                yo = ypool.tile([P, FREE], F16, tag="yout", name=f"yo{c}_{m}")
                nc.vector.tensor_copy(yo, py)
                nc.gpsimd.dma_start(yt[ms, cs], yo)

            # ---------------- one (blk, head) iteration of B -> exp -> D
            def emit_iteration(blk, e, hpar, extra=None):
                bs = slice(blk * BLK, (blk + 1) * BLK)
                head = 2 * e + hpar
                vp16 = vp16A if hpar == 0 else vp16B

                pd = psD.tile([P, BLK], F32, tag="pd", name=f"pd{blk}_{e}_{hpar}")
                pts = {}

                def emit_d(t, first, last):
                    src = pts.pop(t)
                    for j in range(JPB):
                        jo = slice(j * FREE, (j + 1) * FREE)
                        nc.tensor.matmul(
                            pd[:, jo],
                            vp16[:, t, :],
                            src[:, jo],
                            start=first,
                            stop=last,
                            skip_group_check=True,
                        )

                for t in range(NTK):
                    ts_ = slice(t * P, (t + 1) * P)
                    pb = psB.tile([P, BLK], F32, tag="ps", name=f"pb{t}")
                    for j in range(JPB):
                        js = slice(blk * BLK + j * FREE, blk * BLK + (j + 1) * FREE)
                        jo = slice(j * FREE, (j + 1) * FREE)
                        nc.tensor.matmul(
                            pb[:, jo],
                            kc[: HEAD_DIM + 1, ts_],
                            qt65[: HEAD_DIM + 1, head, js],
                        )
                    pts[t] = ptpool.tile([P, BLK], F16, tag="pt", name=f"pt{t}")
                    nc.scalar.activation(
                        pts[t],
                        pb,
                        mybir.ActivationFunctionType.Exp,
                        bias=bshift[:, :],
                        scale=SCALE,
                    )
                    # D for tile t-2 (software-pipelined behind exp)
                    if t >= 2:
                        emit_d(t - 2, first=(t == 2), last=False)
                    if extra and t in extra:
                        extra[t]()
                emit_d(NTK - 2, first=False, last=False)
                emit_d(NTK - 1, first=False, last=True)

                # normalize: out = num * approx(1/denom).
                # reciprocal_approx_fast only works at partition base 0, so
                # for even heads (denominator on PSUM rows 64-127) shift the
                # denominator down via SBUF-SBUF DMA first.
                rec = nrmpool.tile([P, BLK], F32, tag="rec")
                if hpar == 0:
                    dnm = nrmpool.tile([P, BLK], F32, tag="dnm")
                    dnm2 = nrmpool.tile([P, BLK], F32, tag="dnm2")
                    nc.vector.tensor_copy(dnm[HEAD_DIM:, :], pd[HEAD_DIM:, :])
                    q4 = HEAD_DIM // 4
                    for s in range(4):
                        nc.sync.dma_start(
                            dnm2[s * q4 : (s + 1) * q4, :],
                            dnm[HEAD_DIM + s * q4 : HEAD_DIM + (s + 1) * q4, :],
                        )
                    nc.vector.reciprocal_approx_fast(
                        rec[:HEAD_DIM, :], dnm2[:HEAD_DIM, :]
                    )
                    nc.vector.tensor_mul(
                        outs[:HEAD_DIM, e, bs], pd[:HEAD_DIM, :], rec[:HEAD_DIM, :]
                    )
                else:
                    rec2 = nrmpool.tile([P, BLK], F32, tag="rec2")
                    nc.vector.reciprocal_approx_fast(rec[:HEAD_DIM, :], pd[:HEAD_DIM, :])
                    q4 = HEAD_DIM // 4
                    for s in range(4):
                        nc.sync.dma_start(
                            rec2[HEAD_DIM + s * q4 : HEAD_DIM + (s + 1) * q4, :],
                            rec[s * q4 : (s + 1) * q4, :],
                        )
                    nc.vector.tensor_mul(
                        outs[HEAD_DIM:, e, bs], pd[HEAD_DIM:, :], rec2[HEAD_DIM:, :]
                    )

            # ---------------- emission schedule
            emit_kv_chunk(0)
            emit_kv_chunk(1)
            emit_q_chunk(0, 0)
            emit_q_chunk(1, 0)
            emit_v_tiles(0)
            emit_v_tiles(1)

            emit_iteration(
                0, 0, 0,
                extra={
                    2: lambda: emit_q_chunk(0, 1),
                    4: lambda: emit_q_chunk(1, 1),
                    6: lambda: emit_kv_chunk(2),
                    7: lambda: emit_v_tiles(2),
                    8: lambda: emit_kv_chunk(3),
                    9: lambda: emit_v_tiles(3),
                },
            )
            emit_iteration(
                0, 0, 1,
                extra={3: lambda: emit_q_chunk(2, 0), 6: lambda: emit_q_chunk(3, 0)},
            )
            emit_iteration(
                0, 1, 0,
                extra={3: lambda: emit_q_chunk(2, 1), 6: lambda: emit_q_chunk(3, 1)},
            )
            emit_iteration(0, 1, 1)
            # blk1 iterations carry blk0's output projection
            emit_iteration(
                1, 0, 0,
                extra={2 * m + 1: (lambda m=m: emit_out_unit(0, m)) for m in range(MT)},
            )
            emit_iteration(
                1, 0, 1,
                extra={2 * m + 1: (lambda m=m: emit_out_unit(1, m)) for m in range(MT)},
            )
            emit_iteration(1, 1, 0)
            emit_iteration(1, 1, 1)
            for m in range(MT):
                emit_out_unit(2, m)
            for m in range(MT):
                emit_out_unit(3, m)

    nc.finalize()
    return nc


_NC_CACHE = None


def _get_nc():
    global _NC_CACHE
    if _NC_CACHE is None:
        _NC_CACHE = build_bass()
    return _NC_CACHE


def _cid():
    c = np.zeros((P, P + 64), dtype=np.float16)
    c[:, :P] = np.eye(P, dtype=np.float32)
    c[:, P:] = 1.0
    return c


def shard_inputs(query, context, Wq, Wk, Wv, Wo):
    """host-side sharding: 8 cores = batch(2) x kv-group(4)"""
    in_maps = []
    xqT = [np.ascontiguousarray(query[b].T).astype(np.float16) for b in range(B)]
    xcT = [np.ascontiguousarray(context[b].T).astype(np.float16) for b in range(B)]
    for core in range(N_CORES):
        b, g = divmod(core, GROUPS)
        wqT = np.ascontiguousarray(Wq[g * DQ : (g + 1) * DQ, :].T).astype(np.float16)
        wkvT = np.ascontiguousarray(
            np.concatenate(
                [
                    Wk[g * HEAD_DIM : (g + 1) * HEAD_DIM, :],
                    Wv[g * HEAD_DIM : (g + 1) * HEAD_DIM, :],
                ],
                axis=0,
            ).T
        ).astype(np.float16)
        woT = np.ascontiguousarray(Wo[:, g * DQ : (g + 1) * DQ].T).astype(np.float16)
        in_maps.append(
            {
                "xqT": xqT[b],
                "xcT": xcT[b],
                "wqT": wqT,
                "wkvT": wkvT,
                "woT": woT,
                "cid": _cid(),
            }
        )
    return in_maps


def kernel(query, context, Wq, Wk, Wv, Wo, _want_profile=False):
    from concourse.bass_utils import run_bass_kernel_spmd

    nc = _get_nc()
    in_maps = shard_inputs(query, context, Wq, Wk, Wv, Wo)
    res = run_bass_kernel_spmd(
        nc, in_maps, core_ids=list(range(N_CORES)), trace=_want_profile
    )
    out = np.zeros((B, TQ, D_MODEL), dtype=np.float32)
    for core in range(N_CORES):
        b = core // GROUPS
        out[b] += res.results[core]["yT"].T.astype(np.float32)
    if _want_profile:
        return out, res
    return out
